# revision 1
# baseline (speedup 1.0000x reference)
"""Trainium2 Bass kernel for nn_DiscreteAutoregressiveFlow (sampling, forward).

Math: `inputs` is an exact one-hot [B, L, V] tensor. For a row holding token v:
  net = W[v] + b                      (exact: one-hot @ W picks a row)
  loc = one_hot(argmax(net[:V]));  scale = one_hot(argmax(net[V:]))
  one_hot_multiply -> one-hot at (scale_tok*v) % V   (zero row if scale_tok==0)
  one_hot_add      -> one-hot at (scale_tok*v + loc_tok) % V
So out[row] = one_hot(cmap[v]) with a host-precomputed 64-entry map
(sentinel >= V encodes the zero row). The straight-through softmax residuals
and FFT noise in the reference are O(1e-7) and vanish in norm relative error.

Device pipeline per 128x(R*64) chunk (pure streaming, memory-bound):
  xt   = DMA-in (HWDGE, plain)
  prod = xt + cmap/128                (gpsimd tensor_tensor add)
  m    = reduce_max(prod, inner V)    (DVE) = 1 + cmap[tok]/128, exact
  out  = is_equal(1 + iota/128, m)    (DVE) -> one-hot rows, exact 0.0/1.0
  DMA-out (HWDGE)
All f32 values involved are exact (c <= 127 and 2^-7 scaling), so the
comparison is exact. Buffers are fully unrolled per chunk (no WAR waits);
excess waits are legalized by Bacc's generate_event_semaphores.
Sharding: pure data parallel over B*L rows, 8 cores, no collectives.
"""

import numpy as np

V = 64
P = 128
N_CORES = 8
B, L = 16, 8192
ROWS = B * L                      # 131072
ROWS_PER_CORE = ROWS // N_CORES   # 16384
SENTINEL = 100.0
EPS = 1.0 / 128.0

# rows per partition per chunk; chunk = [128, R*64] f32 = R*32KB
R = 16
# Of the 2*n_chunks add/eq ops, how many run on gpsimd (the rest on DVE).
# gpsimd TT is ~2.35x slower per element than DVE TT; ~7/8 adds on gpsimd
# balances the engines (reduce and is_equal are DVE-only at the ISA level).
N_GPSIMD_ADD = 0
N_GPSIMD_EQ = 0

_CACHE = {}


def _build_nc(rows_per_core: int, r: int, n_gp_add: int = N_GPSIMD_ADD,
              n_gp_eq: int = N_GPSIMD_EQ, row_major_partitions: bool = False):
    import concourse.bacc as bacc
    import concourse.bass as bass
    import concourse.mybir as mybir
    from concourse.bass import broadcast_tensor_aps
    from concourse.tile import TileContext

    f32 = mybir.dt.float32
    fd = r * V
    chunk_rows = P * r
    n_chunks = rows_per_core // chunk_rows
    assert rows_per_core % chunk_rows == 0

    # Bacc (not raw Bass): its compile() runs generate_event_semaphores(),
    # which legalizes multi-wait instructions for TRN2 (1 wait per instr).
    nc = bacc.Bacc("TRN2", target_bir_lowering=False, name="daf_onehot")
    x = nc.dram_tensor("x", [rows_per_core, V], f32, kind="ExternalInput")
    cmap = nc.dram_tensor("cmap", [P, V], f32, kind="ExternalInput")
    iota = nc.dram_tensor("iota", [P, V], f32, kind="ExternalInput")
    y = nc.dram_tensor("y", [rows_per_core, V], f32, kind="ExternalOutput")

    if row_major_partitions:
        xv = x.rearrange("(c r p) v -> c p r v", p=P, r=r)
        yv = y.rearrange("(c r p) v -> c p r v", p=P, r=r)
    else:
        xv = x.rearrange("(c p r) v -> c p (r v)", p=P, r=r)
        yv = y.rearrange("(c p r) v -> c p (r v)", p=P, r=r)

    with TileContext(nc) as tc:
        with (
            tc.tile_pool(name="const", bufs=1) as constp,
            tc.tile_pool(name="io", bufs=n_chunks) as iop,
            tc.tile_pool(name="work", bufs=n_chunks) as workp,
        ):
            cmap_st = constp.tile([P, V], f32, tag="cmap_st")
            iota_st = constp.tile([P, V], f32, tag="iota_st")
            nc.sync.dma_start(cmap_st[:], cmap[:])
            nc.sync.dma_start(iota_st[:], iota[:])
            # Each engine reads the constants through its own copy so the
            # hot-loop deps collapse onto that engine's self-semaphore.
            bf16 = mybir.dt.bfloat16
            cmap_1 = cmap_st[:].rearrange("p (o v) -> p o v", o=1)
            iota_1 = iota_st[:].rearrange("p (o v) -> p o v", o=1)
            cmap_f = constp.tile([P, fd], bf16, tag="cmap_f")
            cf3 = cmap_f[:].rearrange("p (r v) -> p r v", v=V)
            cm_b, _ = broadcast_tensor_aps(cmap_1, cf3)
            nc.scalar.copy(cf3, cm_b)
            iota_f = constp.tile([P, fd], bf16, tag="iota_f")
            if3 = iota_f[:].rearrange("p (r v) -> p r v", v=V)
            io_b0, _ = broadcast_tensor_aps(iota_1, if3)
            nc.scalar.copy(if3, io_b0)

            for ci in range(n_chunks):
                # Spread the gpsimd-assigned ops across chunk indices so the
                # two engines interleave rather than serialize phase-wise.
                add_eng = nc.gpsimd if (ci * n_gp_add) % n_chunks < n_gp_add else nc.vector
                eq_eng = nc.gpsimd if (ci * n_gp_eq) % n_chunks < n_gp_eq else nc.vector

                xt = iop.tile([P, fd], f32, tag="x")
                x3 = xt[:].rearrange("p (r v) -> p r v", v=V)
                nc.sync.dma_start(x3 if row_major_partitions else xt[:], xv[ci])

                xb_d = workp.tile([P, fd], bf16, tag="xb_d")
                nc.scalar.copy(xb_d[:], xt[:])
                prod = workp.tile([P, fd], bf16, tag="prod")
                p3 = prod[:].rearrange("p (r v) -> p r v", v=V)
                nc.vector.tensor_tensor(
                    prod[:], xb_d[:], cmap_f[:], op=mybir.AluOpType.add
                )

                c_t = workp.tile([P, r], f32, tag="c")
                nc.vector.tensor_reduce(
                    c_t[:], p3, axis=mybir.AxisListType.X, op=mybir.AluOpType.max
                )

                out_t = iop.tile([P, fd], f32, tag="out")
                o3 = out_t[:].rearrange("p (r v) -> p r v", v=V)
                c3 = c_t[:].rearrange("p (r one) -> p r one", one=1)
                c3_b, _ = broadcast_tensor_aps(c3, o3)
                nc.vector.tensor_tensor(o3, if3, c3_b, op=mybir.AluOpType.is_equal)

                nc.sync.dma_start(yv[ci], o3 if row_major_partitions else out_t[:])

    # Bacc.finalize runs compile(): wait-splitting (generate_event_semaphores),
    # register allocation, nop fusion. run_bass_via_pjrt serializes nc.m as-is,
    # so this must happen here.
    nc.finalize()
    return nc


def _get_nc(rows_per_core=ROWS_PER_CORE, r=R, n_gp_add=N_GPSIMD_ADD,
            n_gp_eq=N_GPSIMD_EQ, row_major_partitions=False):
    key = (rows_per_core, r, n_gp_add, n_gp_eq, row_major_partitions)
    if key not in _CACHE:
        _CACHE[key] = _build_nc(rows_per_core, r, n_gp_add, n_gp_eq,
                                row_major_partitions)
    return _CACHE[key]


def _host_cmap(W: np.ndarray, b: np.ndarray) -> np.ndarray:
    """64-entry map token -> output one-hot index (or sentinel for zero row)."""
    net = W.astype(np.float32) + b.astype(np.float32)[None, :]   # [V, 2V]
    loc_tok = np.argmax(net[:, :V], axis=1)                      # [V]
    scale_tok = np.argmax(net[:, V:], axis=1)                    # [V]
    t = (scale_tok * np.arange(V, dtype=np.int64) + loc_tok) % V
    return np.where(scale_tok == 0, SENTINEL, t.astype(np.float64)).astype(
        np.float32
    )


def _host_tables(W: np.ndarray, b: np.ndarray):
    cmap_eps = _host_cmap(W, b) * np.float32(EPS)                  # exact f32
    iota_eps = 1.0 + np.arange(V, dtype=np.float32) * np.float32(EPS)
    cmap_t = np.tile(cmap_eps.astype(np.float32)[None, :], (P, 1))
    iota_t = np.tile(iota_eps.astype(np.float32)[None, :], (P, 1))
    return cmap_t, iota_t


def kernel(inputs: np.ndarray, W: np.ndarray, b: np.ndarray) -> np.ndarray:
    from concourse import bass_utils

    x = np.ascontiguousarray(inputs.astype(np.float32, copy=False).reshape(ROWS, V))
    cmap_t, iota_t = _host_tables(W, b)

    nc = _get_nc()
    in_maps = [
        {
            "x": x[c * ROWS_PER_CORE : (c + 1) * ROWS_PER_CORE],
            "cmap": cmap_t,
            "iota": iota_t,
        }
        for c in range(N_CORES)
    ]
    res = bass_utils.run_bass_kernel_spmd(nc, in_maps, core_ids=list(range(N_CORES)))
    y = np.concatenate([r["y"] for r in res.results], axis=0)
    return y.reshape(inputs.shape).astype(inputs.dtype, copy=False)



# revision 2
# speedup vs baseline: 1.7937x; 1.7937x over previous
"""Trainium2 Bass kernel for nn_DiscreteAutoregressiveFlow (sampling, forward).

Math: `inputs` is an exact one-hot [B, L, V] tensor. For a row holding token v:
  net = W[v] + b                      (exact: one-hot @ W picks a row)
  loc = one_hot(argmax(net[:V]));  scale = one_hot(argmax(net[V:]))
  one_hot_multiply -> one-hot at (scale_tok*v) % V   (zero row if scale_tok==0)
  one_hot_add      -> one-hot at (scale_tok*v + loc_tok) % V
So out[row] = one_hot(cmap[v]), i.e. out = x @ T for the 0/1 matrix
T[v, cmap[v]] = 1 (zero row for scale_tok==0). The straight-through softmax
residuals and FFT noise in the reference are O(1e-7) and vanish in norm rel
error.

Device pipeline (per core, pure streaming, memory-bound):
  The host uploads x TRANSPOSED and 2-stacked as xs [128, 8192] fp8e4
  (partitions 0:64 = x[:8192].T, 64:128 = x[8192:].T; 0.0/1.0 are exact in
  fp8). The stationary weight is blockdiag(T, T) [128, 128] fp8, loaded from
  DRAM once. Then per N=512 column slice:
    psum[128, 512] f32 = Tbd.T @ xs[:, n:n+512]     (TensorE matmul, exact)
    sbuf fp8 <- psum                                 (ACT/DVE copy-cast)
    DMA out [128, 2048] fp8 chunks
  Host un-transposes and casts fp8 -> f32 (exact for 0/1 values).
HBM traffic per core: 1MB in + 1MB out (vs 8.4MB for f32 I/O).
Sharding: pure data parallel over B*L rows, 8 cores, no collectives.
"""

import numpy as np
import ml_dtypes

V = 64
P = 128
N_CORES = 8
B, L = 16, 8192
ROWS = B * L                      # 131072
ROWS_PER_CORE = ROWS // N_CORES   # 16384
HALF = ROWS_PER_CORE // 2         # 8192 columns in transposed layout

NMM = HALF // 512                 # 16 matmuls of N=512 per core
CHUNK = 2048                      # columns per DMA chunk (256KB fp8)
N_CHUNKS = HALF // CHUNK          # 4 in-DMAs + 4 out-DMAs
MM_PER_CHUNK = CHUNK // 512       # 4

_F8 = ml_dtypes.float8_e4m3

_CACHE = {}


def _build_nc(chunk=CHUNK):
    import concourse.bacc as bacc
    import concourse.mybir as mybir
    from concourse.tile import TileContext

    f8 = mybir.dt.float8e4
    f32 = mybir.dt.float32
    n_chunks = HALF // chunk
    mm_per_chunk = chunk // 512

    # Bacc (not raw Bass): its compile() runs generate_event_semaphores(),
    # which legalizes multi-wait instructions for TRN2 (1 wait per instr).
    nc = bacc.Bacc("TRN2", target_bir_lowering=False, name="daf_mm")
    xs = nc.dram_tensor("xs", [P, HALF], f8, kind="ExternalInput")
    tbd = nc.dram_tensor("tbd", [P, P], f8, kind="ExternalInput")
    y = nc.dram_tensor("y", [P, HALF], f8, kind="ExternalOutput")

    with TileContext(nc) as tc:
        with (
            tc.tile_pool(name="const", bufs=1) as constp,
            tc.tile_pool(name="xin", bufs=n_chunks) as xinp,
            tc.tile_pool(name="yout", bufs=n_chunks) as youtp,
            tc.tile_pool(name="ps", bufs=8, space="PSUM") as psp,
        ):
            t_sb = constp.tile([P, P], f8, tag="t_sb")
            nc.sync.dma_start(t_sb[:], tbd[:])

            for ci in range(n_chunks):
                x_sb = xinp.tile([P, chunk], f8, tag="x")
                nc.sync.dma_start(x_sb[:], xs[:, ci * chunk:(ci + 1) * chunk])

                y_sb = youtp.tile([P, chunk], f8, tag="y")
                for k in range(mm_per_chunk):
                    ps = psp.tile([P, 512], f32, tag="ps")
                    nc.tensor.matmul(
                        ps[:], t_sb[:], x_sb[:, k * 512:(k + 1) * 512],
                        start=True, stop=True,
                    )
                    dst = y_sb[:, k * 512:(k + 1) * 512]
                    # Alternate PSUM->SBUF cast between ACT and DVE so
                    # neither engine bounds the DMA-limited pipeline.
                    if (ci * mm_per_chunk + k) % 2 == 0:
                        nc.scalar.copy(dst, ps[:])
                    else:
                        nc.vector.tensor_copy(dst, ps[:])

                nc.sync.dma_start(y[:, ci * chunk:(ci + 1) * chunk], y_sb[:])

    # Bacc.finalize runs compile(): wait-splitting (generate_event_semaphores),
    # register allocation, nop fusion. run_bass_via_pjrt serializes nc.m as-is,
    # so this must happen here.
    nc.finalize()
    return nc


def _get_nc(chunk=CHUNK):
    if chunk not in _CACHE:
        _CACHE[chunk] = _build_nc(chunk)
    return _CACHE[chunk]


def _host_tmat(W: np.ndarray, b: np.ndarray) -> np.ndarray:
    """[128, 128] fp8 blockdiag(T, T); T[v, cmap[v]] = 1, zero row if
    scale_tok == 0."""
    net = W.astype(np.float32) + b.astype(np.float32)[None, :]   # [V, 2V]
    loc_tok = np.argmax(net[:, :V], axis=1)                      # [V]
    scale_tok = np.argmax(net[:, V:], axis=1)                    # [V]
    t = (scale_tok * np.arange(V, dtype=np.int64) + loc_tok) % V
    T = np.zeros((V, V), dtype=np.float32)
    nz = scale_tok != 0
    T[np.arange(V)[nz], t[nz]] = 1.0
    tbd = np.zeros((P, P), dtype=np.float32)
    tbd[:V, :V] = T
    tbd[V:, V:] = T
    return tbd.astype(_F8)


def _prep_in_maps(inputs: np.ndarray, W: np.ndarray, b: np.ndarray):
    """Shard + transpose-stack + fp8-cast the full one-hot input."""
    x8 = np.asarray(inputs, dtype=np.float32).reshape(ROWS, V).astype(_F8)
    # [cores, 2, HALF, V] -> [cores, 2, V, HALF] -> [cores, 128, HALF]
    xs = np.ascontiguousarray(
        x8.reshape(N_CORES, 2, HALF, V).transpose(0, 1, 3, 2)
    ).reshape(N_CORES, P, HALF)
    tbd = _host_tmat(W, b)
    return [{"xs": xs[c], "tbd": tbd} for c in range(N_CORES)]


def _post(results, dtype, shape):
    yd = np.stack([np.asarray(r["y"]) for r in results])          # [8, 128, HALF]
    y = yd.reshape(N_CORES, 2, V, HALF).transpose(0, 1, 3, 2)     # -> rows, V
    return np.ascontiguousarray(y).reshape(shape).astype(dtype, copy=False)


def kernel(inputs: np.ndarray, W: np.ndarray, b: np.ndarray) -> np.ndarray:
    from concourse import bass_utils

    in_maps = _prep_in_maps(inputs, W, b)
    nc = _get_nc()
    res = bass_utils.run_bass_kernel_spmd(nc, in_maps, core_ids=list(range(N_CORES)))
    return _post(res.results, inputs.dtype, inputs.shape)
